# revision 19
# baseline (speedup 1.0000x reference)
"""Trainium2 Bass kernel for nn_Decoder (GNN message passing):
LSTM(1 step) -> GCNConv -> ReLU -> GCNConv -> Linear -> ReLU on a
100K-node / 1.6M-edge graph, SPMD across 8 NeuronCores.

Strategy (dst-node sharding, norm-factorized):
- Core c owns nodes [c*12500, (c+1)*12500) and all edges into them
  (self-loops included as ordinary edges).
- sym-norm factorization: dinv[src] is folded into the gathered table rows
  (per-partition scale at the transform copy, Scalar engine); dinv[dst] is
  applied per-partition to the node-major aggregation output. This removes
  every per-edge norm multiply on the Vector engine.
- The GCN propagate gathers pre-scaled rows from a bf16 node-major table in
  DRAM (built via sub-AllGathers of the 8 shards, pipelined against the
  per-block transform matmuls) with gpsimd.dma_gather over 4 SWDGE queues,
  then scatter-adds via PE matmul (lhsT = one-hot select, rhs = gathered
  rows -> node-major PSUM accumulators).
- Select matrices are built by is_equal(iota_rep, dst_rep); for most pieces
  dst_rep is materialized on the (otherwise idle) Scalar engine so the DVE
  compare has step-1 operands and hits the 2x packed mode; the rest use the
  broadcast 1x path to balance the two engines.
- Node-major aggregation is re-transposed per block on the PE (identity
  matmul) to keep the feature-major tensor the next matmul needs.
"""

from contextlib import ExitStack

import numpy as np
import ml_dtypes

import concourse.bacc as bacc
import concourse.mybir as mybir
import concourse.tile as tile
from concourse.bass_utils import run_bass_kernel_spmd

P = 128
N = 100000
NCORES = 8
NPC = N // NCORES            # 12500 nodes per core
NBLK = (NPC + P - 1) // P    # 98 dst blocks per core (last has 84)
CH = 4                       # src chunks (int16 gather index limit)
QROWS = NPC // CH            # 3125: per-rank quarter contributed to a chunk
CHROWS = QROWS * NCORES      # 25000 rows per chunk table
GT = 15                      # max tiles (of 128 edges) per dma_gather
GRP = 6                      # dst blocks per PSUM accumulation group
NGRP = (NBLK + GRP - 1) // GRP
MMB = 25                     # transform/bounce blocks per sub-AllGather piece
LSTM_CHUNK = 500             # nodes per LSTM/matmul column chunk
FAST_NUM, FAST_DEN = 5, 8    # fraction of sel builds on the Scalar-assisted
                             # 2x path (rest: DVE broadcast 1x path)

bf16 = ml_dtypes.bfloat16
f32 = np.float32


# ---------------------------------------------------------------- host prep


def _prep_edges(edge_index):
    """Sort/pad each core's incident edges (incl. self-loops) into a
    cross-core-uniform tile schedule. Edges are packed densely per
    (dst-block group, src chunk) SEGMENT; the device runs one matmul per
    (tile, touched block) with per-matmul select columns precomputed here."""
    esrc = np.asarray(edge_index[0], dtype=np.int64)
    edst = np.asarray(edge_index[1], dtype=np.int64)

    deg = np.bincount(edst, minlength=N).astype(np.float64) + 1.0
    dinv = (1.0 / np.sqrt(deg)).astype(np.float32)

    # self-loops are ordinary edges: row dinv[n]*m[n] gathered, post-scale
    # by dinv[dst]=dinv[n] gives the dinv^2 self term exactly
    loop = np.arange(N, dtype=np.int64)
    src = np.concatenate([esrc, loop])
    dst = np.concatenate([edst, loop])

    NSEG = NGRP * CH

    core_of = dst // NPC
    per_core = []
    seg_cnt = np.zeros((NCORES, NSEG), np.int64)
    for c in range(NCORES):
        m = core_of == c
        s = src[m]
        d = dst[m] - c * NPC
        ch = (s % NPC) // QROWS
        o = np.lexsort((d, ch))
        s, d, ch = s[o], d[o], ch[o]
        b = d // P
        sid = (b // GRP) * CH + ch
        seg_cnt[c] = np.bincount(sid, minlength=NSEG)
        per_core.append((s, d, ch, b, sid))

    # tiles per segment: dense packing, padded to the max across cores
    seg_tiles = (seg_cnt.max(axis=0) + P - 1) // P      # [NSEG]
    seg_base = np.concatenate([[0], np.cumsum(seg_tiles)[:-1]])
    TT = int(seg_tiles.sum())
    NIDX = TT * P

    # per-core slot position of each edge + (seg, tile-in-seg, block) triples
    core_pos = []
    touched = [set() for _ in range(NSEG)]              # (tloc, b) per segment
    for c in range(NCORES):
        s, d, ch, b, sid = per_core[c]
        skey = ch * NGRP + (b // GRP)
        cnt_k = np.bincount(skey, minlength=NSEG)
        kstart = np.concatenate([[0], np.cumsum(cnt_k)[:-1]])
        within = np.arange(len(s)) - kstart[skey]
        pos = seg_base[sid] * P + within
        tloc = within // P
        core_pos.append((pos, tloc))
        for ss in range(NSEG):
            msk = sid == ss
            for tb in set(zip(tloc[msk].tolist(), b[msk].tolist())):
                touched[ss].add(tb)

    # matmul schedule: emission order (g, ch, tile, block); pieces of <= GT
    # tiles per gather call with their matmul lists
    mindex = {}                                        # (sid, tloc, b) -> m
    pieces = {}                                        # (g, ch) -> [piece...]
    first_m_of_b = np.full(NBLK, -1, np.int64)
    last_m_of_b = np.full(NBLK, -1, np.int64)
    MT = 0
    GTM = 0
    for g in range(NGRP):
        for ch in range(CH):
            sid = g * CH + ch
            nt = int(seg_tiles[sid])
            t0 = int(seg_base[sid])
            tb_sorted = sorted(touched[sid])
            plist = []
            for k in range(0, nt, GT):
                pn = min(GT, nt - k)
                mlist = []
                m0 = MT
                for tloc, b in tb_sorted:
                    if k <= tloc < k + pn:
                        mindex[(sid, tloc, b)] = MT
                        mlist.append((MT - m0, tloc - k, b))
                        if first_m_of_b[b] < 0:
                            first_m_of_b[b] = MT
                        last_m_of_b[b] = MT
                        MT += 1
                plist.append((t0 + k, pn, m0, mlist))
                GTM = max(GTM, len(mlist))
            pieces[(g, ch)] = plist

    arrs = []
    for c in range(NCORES):
        s, d, ch, b, sid = per_core[c]
        pos, tloc = core_pos[c]

        idxs = np.zeros(NIDX, np.int16)                 # pad -> row 0 (valid)
        # chunk q table = concat over ranks of each rank's q-th quarter
        idxs[pos] = ((s // NPC) * QROWS + (s % QROWS)).astype(np.int16)

        marr = np.fromiter(
            (mindex[(int(ss), int(tt), int(bb))] for ss, tt, bb in zip(sid, tloc, b)),
            dtype=np.int64,
            count=len(s),
        )
        row = pos % P
        dstv = np.full(MT * P, -1.0, np.float32)        # default: no dst match
        dstv[marr * P + row] = (d - b * P).astype(np.float32)

        idx16 = np.tile(np.ascontiguousarray(idxs.reshape(-1, 16).T), (8, 1))
        dstt = np.ascontiguousarray(dstv.reshape(MT, P).T).astype(bf16)
        # per-node dinv, node-major per block: dinv_nm[p, b] = dinv[c*NPC+b*P+p]
        dnm = np.zeros((NBLK, P), np.float32)
        dnm.reshape(-1)[:NPC] = dinv[c * NPC : (c + 1) * NPC]
        dnmt = np.ascontiguousarray(dnm.T)              # [P, NBLK] f32
        # per-node 1/dinv = sqrt(deg), one row (for optional bias preload)
        sqd = np.sqrt(deg[c * NPC : (c + 1) * NPC]).astype(np.float32)
        arrs.append((idx16, dstt, dnmt, sqd.reshape(1, NPC).astype(bf16).copy()))

    sched = dict(
        TT=TT, NIDX=NIDX, MT=MT, GTM=GTM,
        pieces=pieces, first_m_of_b=first_m_of_b, last_m_of_b=last_m_of_b,
    )
    return arrs, sched


# ---------------------------------------------------------------- device


def _build_nc(sched, use_b1, use_b2):
    NIDX = sched["NIDX"]
    MT = sched["MT"]
    GTM = sched["GTM"]
    dt = mybir.dt
    alu = mybir.AluOpType
    act = mybir.ActivationFunctionType

    nc = bacc.Bacc(
        "TRN2",
        target_bir_lowering=False,
        debug=False,
        num_devices=NCORES,
        num_swdge_queues=4,
    )

    # ---- I/O
    zT_d = nc.dram_tensor("zT", [P, NPC], dt.bfloat16, kind="ExternalInput")
    idx_d = nc.dram_tensor("idx16", [P, NIDX // 16], dt.int16, kind="ExternalInput")
    dst_d = nc.dram_tensor("dstv", [P, MT], dt.bfloat16, kind="ExternalInput")
    iotar_d = nc.dram_tensor("iotar", [P, GTM, P], dt.bfloat16, kind="ExternalInput")
    ident_d = nc.dram_tensor("ident", [P, P], dt.bfloat16, kind="ExternalInput")
    dinv_d = nc.dram_tensor("dinvnm", [P, NBLK], dt.float32, kind="ExternalInput")
    wih_d = {
        g: nc.dram_tensor(f"wih_{g}", [P, P], dt.bfloat16, kind="ExternalInput")
        for g in "igo"
    }
    bg_d = {
        g: nc.dram_tensor(f"bg_{g}", [P, 1], dt.float32, kind="ExternalInput")
        for g in "igo"
    }
    w1_d = nc.dram_tensor("w1", [P, P], dt.bfloat16, kind="ExternalInput")
    w2_d = nc.dram_tensor("w2", [P, P], dt.bfloat16, kind="ExternalInput")
    w3t_d = nc.dram_tensor("w3t", [P, P], dt.bfloat16, kind="ExternalInput")
    b3_d = nc.dram_tensor("b3", [P, 1], dt.float32, kind="ExternalInput")
    sqd_d = b1r_d = b2r_d = None
    if use_b1 or use_b2:
        sqd_d = nc.dram_tensor("sqdeg", [1, NPC], dt.bfloat16, kind="ExternalInput")
    if use_b1:
        b1r_d = nc.dram_tensor("b1row", [1, P], dt.bfloat16, kind="ExternalInput")
    if use_b2:
        b2r_d = nc.dram_tensor("b2row", [1, P], dt.bfloat16, kind="ExternalInput")
    out_d = nc.dram_tensor("outT", [P, NPC], dt.float32, kind="ExternalOutput")

    dum_in = nc.dram_tensor("dum_in", [1, P], dt.bfloat16)
    dum_out = nc.dram_tensor("dum_out", [NCORES, P], dt.bfloat16, addr_space="Shared")
    bounce = [nc.dram_tensor(f"bounce{l}", [NPC, P], dt.bfloat16) for l in range(2)]
    table = [
        [
            nc.dram_tensor(
                f"table{l}_{q}", [CHROWS, P], dt.bfloat16, addr_space="Shared"
            )
            for q in range(CH)
        ]
        for l in range(2)
    ]

    with tile.TileContext(nc) as tc, ExitStack() as ctx:
        konst = ctx.enter_context(tc.tile_pool(name="konst", bufs=1))
        big = ctx.enter_context(tc.tile_pool(name="big", bufs=1))

        def load_const(handle, shape, dtype):
            t = konst.tile(shape, dtype, tag=handle.name)
            nc.sync.dma_start(t[:], handle[:])
            return t

        # tiny AllGather issued first: absorbs the one-time collective-init
        # barrier while the LSTM runs
        nc.gpsimd.collective_compute(
            "AllGather",
            mybir.AluOpType.bypass,
            replica_groups=[list(range(NCORES))],
            ins=[dum_in[:]],
            outs=[dum_out[:]],
        )
        ident_t = load_const(ident_d, [P, P], dt.bfloat16)
        dinv_t = load_const(dinv_d, [P, NBLK], dt.float32)
        wih_t = {g: load_const(wih_d[g], [P, P], dt.bfloat16) for g in "igo"}
        bg_t = {g: load_const(bg_d[g], [P, 1], dt.float32) for g in "igo"}
        w1_t = load_const(w1_d, [P, P], dt.bfloat16)
        w2_t = load_const(w2_d, [P, P], dt.bfloat16)
        w3t_t = load_const(w3t_d, [P, P], dt.bfloat16)
        b3_t = load_const(b3_d, [P, 1], dt.float32)
        idx_t = load_const(idx_d, [P, NIDX // 16], dt.int16)
        dst_t = load_const(dst_d, [P, MT], dt.bfloat16)
        iotar_t = load_const(iotar_d, [P, GTM, P], dt.bfloat16)
        sqd_t = load_const(sqd_d, [1, NPC], dt.bfloat16) if sqd_d else None
        b1r_t = load_const(b1r_d, [1, P], dt.bfloat16) if b1r_d else None
        b2r_t = load_const(b2r_d, [1, P], dt.bfloat16) if b2r_d else None

        xT_t = big.tile([P, NPC], dt.bfloat16, tag="xT")  # x1T then x2T

        mm_ps = ctx.enter_context(tc.tile_pool(name="m_ps", bufs=1, space="PSUM"))
        tr_ps = ctx.enter_context(tc.tile_pool(name="tr_ps", bufs=1, space="PSUM"))
        stpool = ctx.enter_context(tc.tile_pool(name="stage", bufs=2))
        xsc = ctx.enter_context(tc.tile_pool(name="xsc", bufs=3))

        # ---------------- phase 1: LSTM -> hT (feature-major, bf16)
        with tc.tile_pool(name="h_pool", bufs=1) as hpool:
            hT_t = hpool.tile([P, NPC], dt.bfloat16, tag="hT")
            with (
                tc.tile_pool(name="lstm_sb", bufs=1) as lsb,
                tc.tile_pool(name="lstm_ps", bufs=6, space="PSUM") as lps,
                tc.tile_pool(name="lstm_tr", bufs=8) as ltr,
            ):
                zT_t = lsb.tile([P, NPC], dt.bfloat16, tag="zT")
                nc.sync.dma_start(zT_t[:], zT_d[:])

                pipe1 = _MMPipe(
                    nc, tc, mm_ps, stpool, hT_t, w1_t, dinv_t, bounce[0], table[0]
                )
                nchunk = (NPC + LSTM_CHUNK - 1) // LSTM_CHUNK
                for k in range(nchunk):
                    c0 = k * LSTM_CHUNK
                    c1 = min(NPC, c0 + LSTM_CHUNK)
                    w = c1 - c0
                    gate = {}
                    for g in "igo":
                        ps = lps.tile([P, LSTM_CHUNK], dt.float32, tag="ps")
                        nc.tensor.matmul(
                            ps[:, :w], wih_t[g][:], zT_t[:, c0:c1], start=True, stop=True
                        )
                        fn = act.Tanh if g == "g" else act.Sigmoid
                        sg = ltr.tile([P, LSTM_CHUNK], dt.bfloat16, tag="sg" + g)
                        nc.scalar.activation(sg[:, :w], ps[:, :w], fn, bias=bg_t[g][:])
                        gate[g] = sg
                    ct = ltr.tile([P, LSTM_CHUNK], dt.bfloat16, tag="ct")
                    nc.vector.tensor_tensor(
                        ct[:, :w], gate["i"][:, :w], gate["g"][:, :w], op=alu.mult
                    )
                    th = ltr.tile([P, LSTM_CHUNK], dt.bfloat16, tag="th")
                    nc.scalar.activation(th[:, :w], ct[:, :w], act.Tanh)
                    nc.vector.tensor_tensor(
                        hT_t[:, c0:c1], gate["o"][:, :w], th[:, :w], op=alu.mult
                    )
                    # phase 2 interleaved: transform blocks fully covered by
                    # the LSTM so sub-AllGathers start during the LSTM sweep
                    pipe1.advance(c1 // P)

            pipe1.advance(NBLK)

        with (
            tc.tile_pool(name="stag", bufs=16) as stag,
            tc.tile_pool(name="selp", bufs=2) as selp,
            tc.tile_pool(name="repp", bufs=2) as repp,
        ):
            # ------------- phase 3: edge layer 1, post: x1 = relu(dinv*agg),
            # node-major -> transpose -> xT; after each group pipe2 advances
            def post1(b, nb, pa):
                xs = xsc.tile([P, P], dt.bfloat16, tag="xs")
                nc.scalar.activation(
                    xs[:nb, :], pa[:nb, :], act.Relu, scale=dinv_t[:nb, b : b + 1]
                )
                tr = tr_ps.tile([P, P], dt.bfloat16, tag="trp")
                nc.tensor.transpose(tr[:, :nb], xs[:nb, :], ident_t[:nb, :nb])
                nc.scalar.activation(
                    xT_t[:, b * P : b * P + nb], tr[:, :nb], act.Copy
                )

            pipe2 = _MMPipe(
                nc, tc, mm_ps, stpool, xT_t, w2_t, dinv_t, bounce[1], table[1]
            )
            _edge_phase(
                nc, tc, table[0], sched, idx_t, dst_t, iotar_t,
                stag, selp, repp, post1,
                sqd_t, b1r_t if use_b1 else None,
                after_group=pipe2.advance,
            )
            pipe2.advance(NBLK)

            # ------------- phase 5: edge layer 2 -> x2 = dinv*agg (no relu)
            def post2(b, nb, pa):
                xs = xsc.tile([P, P], dt.bfloat16, tag="xs")
                nc.scalar.activation(
                    xs[:nb, :], pa[:nb, :], act.Copy, scale=dinv_t[:nb, b : b + 1]
                )
                tr = tr_ps.tile([P, P], dt.bfloat16, tag="trp")
                nc.tensor.transpose(tr[:, :nb], xs[:nb, :], ident_t[:nb, :nb])
                nc.scalar.activation(
                    xT_t[:, b * P : b * P + nb], tr[:, :nb], act.Copy
                )

            _edge_phase(
                nc, tc, table[1], sched, idx_t, dst_t, iotar_t,
                stag, selp, repp, post2,
                sqd_t, b2r_t if use_b2 else None,
            )

        # ---------------- phase 6: outT = relu(W3T.T @ x2T + b3)
        with (
            tc.tile_pool(name="out_ps", bufs=3, space="PSUM") as ops,
            tc.tile_pool(name="out_sb", bufs=3) as osb,
        ):
            nchunk = (NPC + LSTM_CHUNK - 1) // LSTM_CHUNK
            for k in range(nchunk):
                c0 = k * LSTM_CHUNK
                c1 = min(NPC, c0 + LSTM_CHUNK)
                w = c1 - c0
                ps = ops.tile([P, LSTM_CHUNK], dt.float32, tag="ps")
                nc.tensor.matmul(
                    ps[:, :w], w3t_t[:], xT_t[:, c0:c1], start=True, stop=True
                )
                ot = osb.tile([P, LSTM_CHUNK], dt.float32, tag="ot")
                nc.scalar.activation(ot[:, :w], ps[:, :w], act.Relu, bias=b3_t[:])
                nc.sync.dma_start(out_d[:, c0:c1], ot[:, :w])

    nc.compile()
    return nc


class _MMPipe:
    """Per-block transform (featT block @ W -> node-major bf16 rolling stage,
    rows pre-scaled by dinv[node]), with bounce-DMA + sub-AllGather emitted
    per MMB-block piece so the collectives overlap trailing compute."""

    def __init__(self, nc, tc, mm_ps, stpool, featT, w_t, dinv_t, bounce_d, tables_d):
        self.nc = nc
        self.mm_ps = mm_ps
        self.stpool = stpool
        self.featT = featT
        self.w_t = w_t
        self.dinv_t = dinv_t
        self.bounce_d = bounce_d
        self.tables_d = tables_d
        self.stage = None            # rolling per-piece stage tile
        self.done_b = 0
        self.piece = 0

    def advance(self, bend):
        nc = self.nc
        dt = mybir.dt
        act = mybir.ActivationFunctionType
        for b in range(self.done_b, bend):
            nb = min(P, NPC - b * P)
            pc = b // MMB
            blo = pc * MMB
            bhi = min(NBLK, blo + MMB)
            if b == blo:
                self.stage = self.stpool.tile(
                    [P, MMB * P], dt.bfloat16,
                    tag=f"st_{self.bounce_d.name}",
                    name=f"stage_{self.bounce_d.name}_{pc}",
                )
            pm = self.mm_ps.tile([P, P], dt.float32, tag="pm")
            nc.tensor.matmul(
                pm[:nb, :],
                self.featT[:, b * P : b * P + nb],
                self.w_t[:],
                start=True,
                stop=True,
            )
            nc.scalar.activation(
                self.stage[:nb, (b - blo) * P : (b - blo + 1) * P],
                pm[:nb, :],
                act.Copy,
                scale=self.dinv_t[:nb, b : b + 1],
            )
            if b == bhi - 1:
                self._flush(pc, blo, bhi)
        self.done_b = max(self.done_b, bend)

    def _flush(self, p, blo, bhi):
        nc = self.nc
        full = min(bhi * P, (NPC // P) * P)
        nc.sync.dma_start(
            self.bounce_d[blo * P : full, :].rearrange(
                "(b p) f -> p b f", p=P
            ),
            self.stage[:, : full - blo * P].rearrange(
                "p (b f) -> p b f", f=P
            ),
        )
        if bhi * P > full:              # tail remainder rows (12416..12500)
            rem = NPC - full
            lo = full - blo * P
            nc.sync.dma_start(
                self.bounce_d[full:, :],
                self.stage[:rem, lo : lo + P],
            )
        nc.gpsimd.collective_compute(
            "AllGather",
            mybir.AluOpType.bypass,
            replica_groups=[list(range(NCORES))],
            ins=[self.bounce_d[p * QROWS : (p + 1) * QROWS, :]],
            outs=[self.tables_d[p][:]],
        )
        self.stage = None


def _edge_phase(
    nc, tc, table_d, sched, idx_t, dst_t, iotar_t, stag, selp, repp, post,
    sqd_t, brow_t, after_group=None,
):
    dt = mybir.dt
    alu = mybir.AluOpType
    act = mybir.ActivationFunctionType
    pieces = sched["pieces"]
    first_m_of_b = sched["first_m_of_b"]
    last_m_of_b = sched["last_m_of_b"]
    GTM = sched["GTM"]
    use_bias = brow_t is not None
    pidx = 0

    with tc.tile_pool(name="agg_ps", bufs=1, space="PSUM") as aps:
        for g in range(NGRP):
            blo, bhi = g * GRP, min(NBLK, (g + 1) * GRP)
            pa = {}
            for b in range(blo, bhi):
                pa_b = aps.tile(
                    [P, P], dt.float32, tag=f"pa{b - blo}", name=f"pa_{g}_{b}"
                )
                pa[b] = pa_b
                if use_bias:
                    nb = min(P, NPC - b * P)
                    # preload sqrt(deg)[n] * b[f]; post scale by dinv gives +b
                    nc.tensor.matmul(
                        pa_b[:nb, :],
                        sqd_t[0:1, b * P : b * P + nb],
                        brow_t[0:1, :],
                        start=True,
                        stop=False,
                    )
            for ch in range(CH):
                for pt0, pnt, m0, mlist in pieces[(g, ch)]:
                    if not mlist:
                        continue
                    stg = stag.tile([P, GT, P], dt.bfloat16, tag="stag")
                    # split the piece across two SWDGE queues: both halves
                    # drain in parallel rings, halving per-piece latency
                    h = (pnt + 1) // 2
                    nc.gpsimd.dma_gather(
                        stg[:, :h, :],
                        table_d[ch][:],
                        idx_t[:, pt0 * 8 : (pt0 + h) * 8],
                        h * P,
                        h * P,
                        P,
                        single_packet=False,
                        queue_num=ch,
                    )
                    if pnt > h:
                        nc.gpsimd.dma_gather(
                            stg[:, h:pnt, :],
                            table_d[ch][:],
                            idx_t[:, (pt0 + h) * 8 : (pt0 + pnt) * 8],
                            (pnt - h) * P,
                            (pnt - h) * P,
                            P,
                            single_packet=False,
                            queue_num=ch ^ 2,
                        )
                    mn = len(mlist)
                    sel = selp.tile([P, GTM, P], dt.bfloat16, tag="sel")
                    if pidx * FAST_NUM % FAST_DEN < FAST_NUM:
                        # Scalar materializes the broadcast; DVE compares
                        # two step-1 operands (2x packed mode)
                        rep = repp.tile([P, GTM, P], dt.bfloat16, tag="rep")
                        nc.scalar.activation(
                            rep[:, :mn, :],
                            dst_t[:, m0 : m0 + mn]
                            .unsqueeze(2)
                            .broadcast_to([P, mn, P]),
                            act.Copy,
                        )
                        nc.vector.tensor_tensor(
                            sel[:, :mn, :],
                            iotar_t[:, :mn, :],
                            rep[:, :mn, :],
                            op=alu.is_equal,
                        )
                    else:
                        nc.vector.tensor_tensor(
                            sel[:, :mn, :],
                            iotar_t[:, :mn, :],
                            dst_t[:, m0 : m0 + mn]
                            .unsqueeze(2)
                            .broadcast_to([P, mn, P]),
                            op=alu.is_equal,
                        )
                    pidx += 1
                    for mrel, slot, b in mlist:
                        m = m0 + mrel
                        nc.tensor.matmul(
                            pa[b][:],
                            sel[:, mrel, :],
                            stg[:, slot, :],
                            start=(not use_bias) and (m == first_m_of_b[b]),
                            stop=(m == last_m_of_b[b]),
                        )
            for b in range(blo, bhi):
                nb = min(P, NPC - b * P)
                post(b, nb, pa[b])
            if after_group is not None:
                after_group(bhi)


# ---------------------------------------------------------------- entry


def build(z, edge_index, W_ih, W_hh, b_ih, b_hh, W1, b1, W2, b2, W3, b3):
    """Host prep + trace + compile. Returns (nc, in_maps)."""
    z = np.asarray(z, dtype=np.float32)
    W_ih = np.asarray(W_ih, dtype=np.float32)
    b = np.asarray(b_ih, dtype=np.float32) + np.asarray(b_hh, dtype=np.float32)
    b1 = np.asarray(b1, np.float32)
    b2 = np.asarray(b2, np.float32)
    use_b1 = bool(np.any(b1))
    use_b2 = bool(np.any(b2))

    arrs, sched = _prep_edges(edge_index)
    nc = _build_nc(sched, use_b1, use_b2)

    GTM = sched["GTM"]
    iota_rep = np.broadcast_to(
        np.arange(P, dtype=np.float32), (P, GTM, P)
    ).astype(bf16)

    gi = {"i": 0, "g": 2, "o": 3}  # torch gate order i,f,g,o (f unused: c0=0)
    common = {
        "iotar": np.ascontiguousarray(iota_rep),
        "ident": np.eye(P, dtype=np.float32).astype(bf16),
        "w1": np.asarray(W1, np.float32).astype(bf16),
        "w2": np.asarray(W2, np.float32).astype(bf16),
        "w3t": np.ascontiguousarray(np.asarray(W3, np.float32).T).astype(bf16),
        "b3": np.asarray(b3, np.float32).reshape(P, 1).copy(),
    }
    if use_b1:
        common["b1row"] = b1.reshape(1, P).astype(bf16).copy()
    if use_b2:
        common["b2row"] = b2.reshape(1, P).astype(bf16).copy()
    for g, k in gi.items():
        common[f"wih_{g}"] = np.ascontiguousarray(
            W_ih[k * P : (k + 1) * P, :].T
        ).astype(bf16)
        common[f"bg_{g}"] = b[k * P : (k + 1) * P].reshape(P, 1).copy()

    in_maps = []
    for c in range(NCORES):
        idx16, dstt, dnmt, sqd = arrs[c]
        m = dict(common)
        m["zT"] = np.ascontiguousarray(z[c * NPC : (c + 1) * NPC].T).astype(bf16)
        m["idx16"] = idx16
        m["dstv"] = dstt
        m["dinvnm"] = dnmt
        if use_b1 or use_b2:
            m["sqdeg"] = sqd
        in_maps.append(m)
    return nc, in_maps


def assemble(results):
    out = np.empty((N, P), np.float32)
    for c in range(NCORES):
        out[c * NPC : (c + 1) * NPC] = results[c]["outT"].T
    return out


def kernel(z, edge_index, W_ih, W_hh, b_ih, b_hh, W1, b1, W2, b2, W3, b3):
    nc, in_maps = build(z, edge_index, W_ih, W_hh, b_ih, b_hh, W1, b1, W2, b2, W3, b3)
    res = run_bass_kernel_spmd(nc, in_maps, core_ids=list(range(NCORES)))
    return assemble(res.results)
